# revision 1
# baseline (speedup 1.0000x reference)
"""EncoderBlock kernel for 8 Trainium2 NeuronCores (data-parallel over batch).

Contract: kernel(**inputs) takes the FULL inputs of reference.setup_inputs()
and returns the FULL [16, 1024, 768] float32 output.

Strategy: pure data parallelism — 16 batches / 8 cores = 2 batches per core,
weights replicated, zero collectives. Per core a fused Bass/Tile program runs
LN1 -> QKV -> attention -> proj -> residual -> LN2 -> fc1/gelu -> fc2 ->
(normalized residual) -> LN3. Matmuls use float32r (full PE rate at
moving-dim >= 256, ~1e-4 rounding). LN1 gamma/beta fold into the QKV
weights/bias on the host; LN2/LN3 gamma/beta are applied on device.

Attention layout: Q^T,K^T are produced feature-major ([64, n] per head,
head pairs stacked on partitions), scores^T = K_h^T.T @ Q_h^T lands
keys-major so softmax needs no transposes; exp goes through the scalar
engine; P @ V uses a packed V tile ([V_h | ones] in a 128-wide, parity-
offset layout) so the softmax denominator rides along on an aligned spare
partition, then a K=1 matmul broadcasts 1/denom for the normalize.
"""

import os
import sys

sys.path.insert(0, "/opt/trn_rl_repo")
# The axon NTFF profiling hook is unavailable in this environment; force
# tracing off so an externally-set BASS_TRACE cannot break execution.
os.environ["BASS_NEVER_TRACE"] = "1"

import numpy as np

import concourse.bass as bass
import concourse.tile as tile
from concourse import mybir
from concourse.masks import make_identity
from concourse.vector_clock import ScopedClock, VectorClock
from concourse.bass_utils import run_bass_kernel_spmd

F32 = mybir.dt.float32
F32R = mybir.dt.float32r
AF = mybir.ActivationFunctionType
ALU = mybir.AluOpType

B, N, D = 16, 1024, 768
H, DH, HID = 12, 64, 3072
NCORES = 8
BLOC = B // NCORES
EPS = 1e-5
TC_N = N // 128   # 8 token tiles / batch
KC_D = D // 128   # 6 feature chunks
MC_H = HID // 128  # 24 hidden chunks


# ---------------------------------------------------------------------------
# Workarounds: this walrus build rejects >1 sync-wait command per instruction.
# ---------------------------------------------------------------------------
def _patched_drain_and_barrier(self, tick_clock, wait_clock):
    gc = tick_clock.global_clock
    n = len(gc)
    for i in range(n):
        t = gc[i]
        if t <= 0:
            continue
        vec = [0] * n
        vec[i] = t
        nop = self.nc.sync.nop(nofuse=True)
        wait_clock.add_sem_waits(nop.ins, ScopedClock({None: VectorClock(vec)}))
    self.nc.sync.drain()
    self.nc.all_engine_barrier()
    assert self.sems is not None
    popped = self.nc._tile_sem_poison_stack.pop()
    assert popped is self._sem_poison
    self.nc.clear_and_free_semaphores(list(self.sems.allocated().values()))
    self.nc.all_engine_barrier()


tile.TileContext._drain_and_barrier = _patched_drain_and_barrier


def _split_sync_waits(nc, limit=1):
    """Move excess per-instruction sync waits onto same-engine NoOps."""
    n_split = 0
    for fn in nc.m.functions:
        for bb in fn.blocks:
            out = []
            for ins in bb.instructions:
                si = ins.sync_info
                waits = list(si.on_wait) if (si and si.on_wait) else []
                if len(waits) > limit:
                    excess, keep = waits[:-limit], waits[-limit:]
                    for w in excess:
                        nop = mybir.InstNoOp(
                            name=f"{ins.name}-ws{n_split}",
                            engine=ins.engine,
                            ins=[],
                            outs=[],
                            sync_info=mybir.SyncInfo(on_wait=[w], on_update=[]),
                        )
                        n_split += 1
                        out.append(nop)
                    si.on_wait = keep
                out.append(ins)
            bb.instructions = out
    return n_split


# ---------------------------------------------------------------------------
# Device program (one core's share: BLOC batches)
# ---------------------------------------------------------------------------
def _layer_norm(nc, misc, x_t, eps_t):
    """(mean, rstd) [128,1] via DVE bn_stats for x_t [128, 768] f32."""
    xr = x_t.rearrange("p (s d) -> p s d", d=256)
    lt = misc.tile([128, 24], F32, tag="lnb")
    stats = lt[:, 0:18].rearrange("p (s d) -> p s d", d=6)
    for s in range(3):
        nc.vector.bn_stats(out=stats[:, s, :], in_=xr[:, s, :])
    mv = lt[:, 18:20]
    nc.vector.bn_aggr(out=mv, in_=lt[:, 0:18])
    std = lt[:, 20:21]
    nc.scalar.activation(out=std, in_=mv[:, 1:2], func=AF.Sqrt, bias=eps_t)
    rstd = misc.tile([128, 1], F32, tag="rstd")
    nc.vector.reciprocal(out=rstd, in_=std)
    return mv[:, 0:1], rstd


def _ln_stats(nc, misc, x_f32, eps_t, sumx=None):
    """(mean, rstd) [128,1] for x_f32 [128, 768].  If sumx is None it is
    computed here via an ACT Identity pass; Sum(x^2) always comes from an
    ACT Square pass; var = E[x^2] - mean^2 with the sign folded into Sqrt."""
    scr = misc.tile([128, D], F32, tag="scr")
    if sumx is None:
        sumx = misc.tile([128, 1], F32, tag="sumx")
        nc.scalar.activation(out=scr, in_=x_f32, func=AF.Identity,
                             accum_out=sumx)
    ssq = misc.tile([128, 1], F32, tag="ssq")
    scr2 = misc.tile([128, D], F32, tag="scr")
    nc.scalar.activation(out=scr2, in_=x_f32, func=AF.Square, accum_out=ssq)
    lt = misc.tile([128, 4], F32, tag="ln")
    mean = lt[:, 0:1]
    nc.vector.tensor_scalar(out=mean, in0=sumx, scalar1=1.0 / D, scalar2=None,
                            op0=ALU.mult)
    exsq = lt[:, 1:2]
    nc.vector.tensor_scalar(out=exsq, in0=ssq, scalar1=1.0 / D, scalar2=None,
                            op0=ALU.mult)
    negvar = lt[:, 2:3]
    nc.vector.scalar_tensor_tensor(out=negvar, in0=mean, scalar=mean, in1=exsq,
                                   op0=ALU.mult, op1=ALU.subtract)
    std = lt[:, 3:4]
    nc.scalar.activation(out=std, in_=negvar, func=AF.Sqrt, bias=eps_t,
                         scale=-1.0)
    rstd = misc.tile([128, 1], F32, tag="rstd")
    nc.vector.reciprocal(out=rstd, in_=std)
    return mean, rstd


def _build_nc(reps=1, skip_gb2=False, skip_gb3=False):
    nc = bass.Bass()

    x_d = nc.dram_tensor("x", [BLOC, N, D], F32R, kind="ExternalInput")
    wqk_d = nc.dram_tensor("w_qk", [D, 2 * D], F32R, kind="ExternalInput")
    wv_d = nc.dram_tensor("w_v", [D, D], F32R, kind="ExternalInput")
    bqk_d = nc.dram_tensor("b_qk", [2 * D], F32, kind="ExternalInput")
    bv_d = nc.dram_tensor("b_v", [D], F32, kind="ExternalInput")
    pw_d = nc.dram_tensor("p_w", [D, D], F32R, kind="ExternalInput")
    pbr_d = nc.dram_tensor("pbr", [D], F32R, kind="ExternalInput")
    w1_d = nc.dram_tensor("w1", [D, HID], F32R, kind="ExternalInput")
    b1_d = nc.dram_tensor("b1", [HID], F32, kind="ExternalInput")
    w2_d = nc.dram_tensor("w2", [HID, D], F32R, kind="ExternalInput")
    b2fr_d = nc.dram_tensor("b2fr", [D], F32R, kind="ExternalInput")
    g2_d = nc.dram_tensor("g2", [D], F32, kind="ExternalInput")
    bt2_d = nc.dram_tensor("bt2", [D], F32, kind="ExternalInput")
    g3_d = nc.dram_tensor("g3", [D], F32, kind="ExternalInput")
    bt3_d = nc.dram_tensor("bt3", [D], F32, kind="ExternalInput")
    y_d = nc.dram_tensor("y", [BLOC, N, D], F32, kind="ExternalOutput")
    x2s_d = nc.dram_tensor("x2s", [BLOC, N, D], F32)

    with tile.TileContext(nc, pool_alloc_mode="queue") as tc:
        misc = tc.alloc_tile_pool(name="misc", bufs=2)
        const = tc.alloc_tile_pool(name="const", bufs=1)

        ident = const.tile([128, 128], F32)
        make_identity(nc, ident)
        eps_t = const.tile([128, 1], F32)
        nc.vector.memset(eps_t, EPS)
        ones_row = const.tile([1, 128], F32R)
        nc.vector.memset(ones_row.bitcast(F32), 1.0)
        pbr_t = const.tile([1, D], F32R)
        nc.gpsimd.dma_start(out=pbr_t, in_=pbr_d[None, :])
        b2fr_t = const.tile([1, D], F32R)
        nc.gpsimd.dma_start(out=b2fr_t, in_=b2fr_d[None, :])
        onec = const.tile([128, TC_N, 6, 1], F32)
        nc.vector.memset(onec, 1.0)
        bqk_t = const.tile([128, 12], F32)
        nc.scalar.dma_start(out=bqk_t, in_=bqk_d.rearrange("(c p) -> p c", p=128))
        b1_t = const.tile([128, MC_H], F32)
        nc.scalar.dma_start(out=b1_t, in_=b1_d.rearrange("(c p) -> p c", p=128))
        def load_bc(pool, dd, nm):
            t = pool.tile([128, D], F32, tag=nm, name=nm)
            nc.sync.dma_start(out=t, in_=dd[None, :].partition_broadcast(128))
            return t

        for rep in range(reps):
          for b0 in range(BLOC):
            b = b0  # DRAM indices unchanged across reps
            rb = rep * BLOC + b0  # unique pool names
            # Pool lifetimes are strictly nested (stack discipline):
            # arena [start..E1] > pxnt [start..C] > phase pools
            arena = tc.alloc_tile_pool(name=f"arena{rb}", bufs=1)

            # ---- Phase A: LN1 + fused normalize/transpose -> xnT ----
            # xnT = x^T @ diag(rstd) + ones_col @ (-mean*rstd)^T, done on PE.
            pxnt = tc.alloc_tile_pool(name=f"pxnt{rb}", bufs=1)
            xnT = pxnt.tile([128, KC_D, N], F32R)
            psA = tc.alloc_tile_pool(name=f"psA{rb}", bufs=4, space="PSUM")
            for tcn in range(TC_N):
                x_t = misc.tile([128, D], F32R, tag="x_in", bufs=3)
                nc.sync.dma_start(out=x_t, in_=x_d[b, tcn * 128:(tcn + 1) * 128, :])
                xf = x_t.bitcast(F32)
                mean, rstd = _layer_norm(nc, misc, xf, eps_t)
                nmr = misc.tile([128, 1], F32, tag="nmr")
                nc.vector.tensor_scalar(out=nmr, in0=mean, scalar1=rstd,
                                        scalar2=-1.0, op0=ALU.mult, op1=ALU.mult)
                prow = psA.tile([1, 128], F32, tag="prow")
                nc.tensor.transpose(prow, nmr, ident)
                nmr_row = misc.tile([1, 128], F32R, tag="nmr_row")
                nc.vector.tensor_copy(out=nmr_row, in_=prow)
                diag_r = misc.tile([128, 128], F32R, tag="diag_r")
                nc.vector.tensor_scalar_mul(out=diag_r, in0=ident, scalar1=rstd)
                for kc in range(KC_D):
                    pt = psA.tile([128, 128], F32, tag="tp")
                    nc.tensor.matmul(pt, x_t[:, kc * 128:(kc + 1) * 128], diag_r,
                                     start=True, stop=False)
                    nc.tensor.matmul(pt, ones_row, nmr_row,
                                     start=False, stop=True)
                    nc.vector.tensor_copy(
                        out=xnT[:, kc, tcn * 128:(tcn + 1) * 128], in_=pt)
            psA.release()

            # ---- Phase C: V projection (token-major, packed [V_h | ones]) ----
            # per (token-chunk, head): even head -> V cols 0:64, ones col 64;
            # odd head -> ones col 0, V cols 64:128.  Ctx then lands on
            # partitions (h%2)*64..+64 and the denom on an aligned partition.
            V1 = arena.tile([128, TC_N, H, 128], F32R, tag="V1")
            # init only the zero/ones regions; V regions are written by the
            # projection below (disjoint ranges -> no WAW serialization)
            nc.gpsimd.memset(V1[:, :, 0::2, 65:128].bitcast(F32), 0.0)
            nc.gpsimd.memset(V1[:, :, 1::2, 1:64].bitcast(F32), 0.0)
            nc.gpsimd.tensor_copy(out=V1[:, :, 0::2, 64:65], in_=onec)
            nc.gpsimd.tensor_copy(out=V1[:, :, 1::2, 0:1], in_=onec)
            pwv = tc.alloc_tile_pool(name=f"pwv{rb}", bufs=1)
            bv_t = load_bc(pxnt, bv_d, f"bv{rb}")
            wv = pwv.tile([128, KC_D, D], F32R)
            wv_r = wv_d.rearrange("(c p) n -> p c n", p=128)
            for kc in range(KC_D):
                eng = nc.sync if kc % 2 == 0 else nc.scalar
                eng.dma_start(out=wv[:, kc, :], in_=wv_r[:, kc, :])
            psC = tc.alloc_tile_pool(name=f"psC{rb}", bufs=3, space="PSUM")
            for tcn in range(TC_N):
                ps = psC.tile([128, D], F32, tag="v")
                for kc in range(KC_D):
                    lhsT = xnT[:, kc, tcn * 128:(tcn + 1) * 128]
                    nc.tensor.matmul(ps[:, 0:512], lhsT, wv[:, kc, 0:512],
                                     start=(kc == 0), stop=(kc == KC_D - 1))
                    nc.tensor.matmul(ps[:, 512:768], lhsT, wv[:, kc, 512:768],
                                     start=(kc == 0), stop=(kc == KC_D - 1))
                pv3 = ps.rearrange("p (h d) -> p h d", d=DH)
                bv3 = bv_t.rearrange("p (h d) -> p h d", d=DH)
                nc.vector.tensor_tensor(
                    out=V1[:, tcn, 0::2, 0:64], in0=pv3[:, 0::2, :],
                    in1=bv3[:, 0::2, :], op=ALU.add)
                nc.vector.tensor_tensor(
                    out=V1[:, tcn, 1::2, 64:128], in0=pv3[:, 1::2, :],
                    in1=bv3[:, 1::2, :], op=ALU.add)
            psC.release()
            pwv.release()

            # ---- Phase B: Q,K projection (feature-major) ----
            qkT = arena.tile([128, 12, N], F32R, tag="qkT")
            pwqk = tc.alloc_tile_pool(name=f"pwqk{rb}", bufs=1)
            psB = tc.alloc_tile_pool(name=f"psB{rb}", bufs=4, space="PSUM")
            for half in range(2):  # Q columns then K columns of w_qk
                wqk = pwqk.tile([128, KC_D, D], F32R, tag="wqk")
                wqk_r = wqk_d[:, half * D:(half + 1) * D].rearrange(
                    "(c p) n -> p c n", p=128)
                for kc in range(KC_D):
                    eng = nc.sync if kc % 2 == 0 else nc.scalar
                    eng.dma_start(out=wqk[:, kc, :], in_=wqk_r[:, kc, :])
                for fc6 in range(6):
                    fc = half * 6 + fc6
                    for nh in range(2):
                        ps = psB.tile([128, 512], F32, tag="qk")
                        for kc in range(KC_D):
                            nc.tensor.matmul(
                                ps,
                                wqk[:, kc, fc6 * 128:(fc6 + 1) * 128],
                                xnT[:, kc, nh * 512:(nh + 1) * 512],
                                start=(kc == 0), stop=(kc == KC_D - 1))
                        nc.vector.tensor_scalar(
                            out=qkT[:, fc, nh * 512:(nh + 1) * 512], in0=ps,
                            scalar1=bqk_t[:, fc:fc + 1], scalar2=None, op0=ALU.add)
            psB.release()
            pwqk.release()
            pxnt.release()

            # ---- Phase D: attention ----
            ctxT = arena.tile([128, KC_D, N], F32R, tag="ctxT", name="ctxT")
            pPT = tc.alloc_tile_pool(name=f"pPT{rb}", bufs=2)
            psS = tc.alloc_tile_pool(name=f"psS{rb}", bufs=2, space="PSUM")
            psX = tc.alloc_tile_pool(name=f"psX{rb}", bufs=2, space="PSUM")
            for nh in range(2):
                for h in range(H):
                    qh = (h % 2) * 64
                    dh = 64 if h % 2 == 0 else 0  # denominator partition
                    fq, fk = h // 2, 6 + h // 2
                    PT = pPT.tile([128, TC_N, 512], F32R, tag="PT", bufs=2)
                    for mc2 in range(TC_N // 2):
                        sp = psS.tile([128, 1024], F32, tag="sc")
                        for mi in range(2):
                            mc = mc2 * 2 + mi
                            nc.tensor.matmul(
                                sp[:, mi * 512:(mi + 1) * 512],
                                qkT[qh:qh + 64, fk, mc * 128:(mc + 1) * 128],
                                qkT[qh:qh + 64, fq, nh * 512:(nh + 1) * 512],
                                start=True, stop=True)
                        nc.scalar.activation(
                            out=PT[:, mc2 * 2:mc2 * 2 + 2, :], in_=sp,
                            func=AF.Exp, scale=0.125)
                    cp = psX.tile([128, 512], F32, tag="cp")
                    for mc in range(TC_N):
                        nc.tensor.matmul(cp, V1[:, mc, h, :], PT[:, mc, :],
                                         start=(mc == 0), stop=(mc == TC_N - 1))
                    rd = misc.tile([1, 512], F32R, tag="rd", name="rd")
                    with nc.allow_low_precision(reason="f32r is fp32-width"):
                        nc.vector.reciprocal(out=rd, in_=cp[dh:dh + 1, :])
                    bb = psX.tile([128, 512], F32, tag="bb")
                    nc.tensor.matmul(bb, ones_row, rd, start=True, stop=True)
                    bsb = misc.tile([128, 512], F32, tag="scr", name="bsb")
                    nc.vector.tensor_copy(out=bsb[qh:qh + 64, :], in_=bb[qh:qh + 64, :])
                    nc.vector.tensor_tensor(
                        out=ctxT[qh:qh + 64, fq, nh * 512:(nh + 1) * 512],
                        in0=cp[qh:qh + 64, :], in1=bsb[qh:qh + 64, :],
                        op=ALU.mult)
            psX.release()
            psS.release()
            pPT.release()

            # ---- Phase E1: proj + residual + LN2 -> DRAM bounce (ff_in) ----
            ppw = tc.alloc_tile_pool(name=f"ppw{rb}", bufs=1)
            pw = ppw.tile([128, KC_D, D], F32R)
            pw_r = pw_d.rearrange("(c p) n -> p c n", p=128)
            for dc in range(KC_D):
                eng = nc.sync if dc % 2 == 0 else nc.scalar
                eng.dma_start(out=pw[:, dc, :], in_=pw_r[:, dc, :])
            psE = tc.alloc_tile_pool(name=f"psE{rb}", bufs=3, space="PSUM")
            pbeg = tc.alloc_tile_pool(name=f"pbeg{rb}", bufs=1)
            if not skip_gb2:
                g2_t = load_bc(pbeg, g2_d, f"g2{rb}")
                bt2_t = load_bc(pbeg, bt2_d, f"bt2{rb}")
            for tcn in range(TC_N):
                ps = psE.tile([128, D], F32, tag="pj")
                x_t = misc.tile([128, D], F32R, tag="x_in", bufs=3)
                nc.scalar.dma_start(out=x_t, in_=x_d[b, tcn * 128:(tcn + 1) * 128, :])
                x2 = misc.tile([128, D], F32, tag="xwork", bufs=3)
                xf = x_t.bitcast(F32)
                for dc in range(KC_D):
                    nc.tensor.matmul(ps[:, 0:512],
                                     ctxT[:, dc, tcn * 128:(tcn + 1) * 128],
                                     pw[:, dc, 0:512],
                                     start=(dc == 0), stop=False)
                nc.tensor.matmul(ps[:, 0:512], ones_row, pbr_t[:, 0:512],
                                 start=False, stop=True)
                nc.vector.tensor_tensor(out=x2[:, 0:512], in0=ps[:, 0:512],
                                        in1=xf[:, 0:512], op=ALU.add)
                for dc in range(KC_D):
                    nc.tensor.matmul(ps[:, 512:768],
                                     ctxT[:, dc, tcn * 128:(tcn + 1) * 128],
                                     pw[:, dc, 512:768],
                                     start=(dc == 0), stop=False)
                nc.tensor.matmul(ps[:, 512:768], ones_row, pbr_t[:, 512:768],
                                 start=False, stop=True)
                nc.vector.tensor_tensor(out=x2[:, 512:768], in0=ps[:, 512:768],
                                        in1=xf[:, 512:768], op=ALU.add)
                mean, rstd = _layer_norm(nc, misc, x2, eps_t)
                fi = misc.tile([128, D], F32, tag="xout")
                nc.vector.tensor_scalar(out=fi, in0=x2, scalar1=mean, scalar2=rstd,
                                        op0=ALU.subtract, op1=ALU.mult)
                if not skip_gb2:
                    nc.gpsimd.tensor_tensor(out=fi, in0=fi, in1=g2_t, op=ALU.mult)
                    nc.gpsimd.tensor_tensor(out=fi, in0=fi, in1=bt2_t, op=ALU.add)
                nc.scalar.dma_start(out=x2s_d[b, tcn * 128:(tcn + 1) * 128, :], in_=fi)
            psE.release()
            pbeg.release()
            ppw.release()
            arena.release()

            # ---- Phase E2: LN2 + gamma/beta + transpose ----
            pff = tc.alloc_tile_pool(name=f"pff{rb}", bufs=1)
            pffh = tc.alloc_tile_pool(name=f"pffh{rb}", bufs=1)
            pft = tc.alloc_tile_pool(name=f"pft{rb}", bufs=1)
            ff_in = pff.tile([128, TC_N, D], F32)
            ffinT = pft.tile([128, KC_D, N], F32R)
            psT = tc.alloc_tile_pool(name=f"psT{rb}", bufs=4, space="PSUM")
            for tcn in range(TC_N):
                fi = ff_in[:, tcn, :]
                nc.scalar.dma_start(out=fi, in_=x2s_d[b, tcn * 128:(tcn + 1) * 128, :])
                for kc in range(KC_D):
                    pt = psT.tile([128, 128], F32, tag="tp2")
                    nc.tensor.transpose(pt, fi[:, kc * 128:(kc + 1) * 128], ident)
                    nc.vector.tensor_copy(
                        out=ffinT[:, kc, tcn * 128:(tcn + 1) * 128], in_=pt)
            psT.release()

            # ---- Phases F/G interleaved in n-halves: fc1+gelu for a
            # 512-token half, then fc2+residual+LN3 for those tokens. ----
            ffhT = pffh.tile([128, MC_H, 512], F32R)
            pw1 = tc.alloc_tile_pool(name=f"pw1{rb}", bufs=5)
            pw2 = tc.alloc_tile_pool(name=f"pw2{rb}", bufs=4)
            pbe3 = tc.alloc_tile_pool(name=f"pbe3{rb}", bufs=1)
            if not skip_gb3:
                g3_t = load_bc(pbe3, g3_d, f"g3{rb}")
                bt3_t = load_bc(pbe3, bt3_d, f"bt3{rb}")
            for nh in range(2):
                psF = tc.alloc_tile_pool(name=f"psF{rb}_{nh}", bufs=4,
                                         space="PSUM")
                for mcg in range(MC_H // 2):
                    w1b = pw1.tile([128, KC_D, 256], F32R, tag="w1b")
                    w1r = w1_d[:, mcg * 256:(mcg + 1) * 256].rearrange(
                        "(c p) n -> p c n", p=128)
                    for kc in range(KC_D):
                        weng = nc.sync if (mcg + kc) % 2 == 0 else nc.scalar
                        weng.dma_start(out=w1b[:, kc, :], in_=w1r[:, kc, :])
                    for mi in range(2):
                        mc = mcg * 2 + mi
                        ps = psF.tile([128, 512], F32, tag="f1")
                        for kc in range(KC_D):
                            nc.tensor.matmul(
                                ps,
                                w1b[:, kc, mi * 128:(mi + 1) * 128],
                                ffinT[:, kc, nh * 512:(nh + 1) * 512],
                                start=(kc == 0), stop=(kc == KC_D - 1))
                        nc.scalar.activation(
                            out=ffhT[:, mc, :], in_=ps,
                            func=AF.Gelu, bias=b1_t[:, mc:mc + 1])
                psF.release()
                psG = tc.alloc_tile_pool(name=f"psG{rb}_{nh}", bufs=4,
                                         space="PSUM")
                pss = [psG.tile([128, D], F32, tag="f2", name=f"f2_{rb}_{nh}_{i}")
                       for i in range(4)]
                for kc in range(MC_H):
                    w2t = pw2.tile([128, D], F32R, tag="w2t")
                    weng2 = nc.sync if kc % 2 == 0 else nc.scalar
                    weng2.dma_start(
                        out=w2t, in_=w2_d[kc * 128:(kc + 1) * 128, :])
                    for ti in range(4):
                        lhsT = ffhT[:, kc, ti * 128:(ti + 1) * 128]
                        nc.tensor.matmul(pss[ti][:, 0:512], lhsT, w2t[:, 0:512],
                                         start=(kc == 0), stop=False)
                        nc.tensor.matmul(pss[ti][:, 512:768], lhsT,
                                         w2t[:, 512:768],
                                         start=(kc == 0), stop=False)
                for ti in range(4):
                    nc.tensor.matmul(pss[ti][:, 0:512], ones_row,
                                     b2fr_t[:, 0:512], start=False, stop=True)
                    nc.tensor.matmul(pss[ti][:, 512:768], ones_row,
                                     b2fr_t[:, 512:768], start=False, stop=True)
                for ti in range(4):
                    tcn = nh * 4 + ti
                    x3 = misc.tile([128, D], F32, tag="xwork", bufs=3)
                    nc.vector.tensor_tensor(out=x3, in0=pss[ti],
                                            in1=ff_in[:, tcn, :], op=ALU.add)
                    mean, rstd = _layer_norm(nc, misc, x3, eps_t)
                    yt = misc.tile([128, D], F32, tag="xout")
                    neng = nc.vector if ti % 2 == 0 else nc.gpsimd
                    neng.tensor_scalar(out=yt, in0=x3, scalar1=mean,
                                       scalar2=rstd,
                                       op0=ALU.subtract, op1=ALU.mult)
                    if not skip_gb3:
                        nc.gpsimd.tensor_tensor(out=yt, in0=yt, in1=g3_t, op=ALU.mult)
                        nc.gpsimd.tensor_tensor(out=yt, in0=yt, in1=bt3_t, op=ALU.add)
                    nc.scalar.dma_start(out=y_d[b, tcn * 128:(tcn + 1) * 128, :],
                                      in_=yt)
                psG.release()
            pbe3.release()
            pw2.release()
            pw1.release()
            pft.release()
            pffh.release()
            pff.release()

        const.release()
        misc.release()

    _split_sync_waits(nc)
    return nc


_NC_CACHE = {}


def _get_nc(reps=1, skip_gb2=False, skip_gb3=False):
    key = f"nc{reps}_{skip_gb2}_{skip_gb3}"
    if key not in _NC_CACHE:
        _NC_CACHE[key] = _build_nc(reps, skip_gb2, skip_gb3)
    return _NC_CACHE[key]


def kernel(x, ln1_g, ln1_b, qkv_w, qkv_b, proj_w, proj_b,
           ln2_g, ln2_b, fc1_w, fc1_b, fc2_w, fc2_b, ln3_g, ln3_b,
           **extra):
    x = np.asarray(x, np.float32)
    f = lambda a: np.ascontiguousarray(np.asarray(a, np.float32))
    qkv_w, qkv_b = f(qkv_w), f(qkv_b)

    # Fold LN1 gamma/beta into QKV weights/bias (host, fp32).
    w_eff = np.asarray(ln1_g, np.float32)[:, None] * qkv_w
    b_eff = np.asarray(ln1_b, np.float32) @ qkv_w + qkv_b

    common = {
        "w_qk": f(w_eff[:, :2 * D]),
        "w_v": f(w_eff[:, 2 * D:]),
        "b_qk": f(b_eff[:2 * D]),
        "b_v": f(b_eff[2 * D:]),
        "p_w": f(proj_w), "pbr": f(proj_b),
        "w1": f(fc1_w), "b1": f(fc1_b),
        "w2": f(fc2_w), "b2fr": f(fc2_b),
        "g2": f(ln2_g), "bt2": f(ln2_b),
        "g3": f(ln3_g), "bt3": f(ln3_b),
    }
    in_maps = [dict(common, x=f(x[i * BLOC:(i + 1) * BLOC])) for i in range(NCORES)]

    skip_gb2 = bool(np.all(common["g2"] == 1.0) and np.all(common["bt2"] == 0.0))
    skip_gb3 = bool(np.all(common["g3"] == 1.0) and np.all(common["bt3"] == 0.0))
    nc = _get_nc(1, skip_gb2, skip_gb3)
    res = run_bass_kernel_spmd(nc, in_maps, core_ids=list(range(NCORES)))
    _NC_CACHE["last_result"] = res
    return np.concatenate([r["y"] for r in res.results], axis=0)



# revision 7
# speedup vs baseline: 1.5446x; 1.5446x over previous
"""EncoderBlock kernel for 8 Trainium2 NeuronCores (data-parallel over batch).

Contract: kernel(**inputs) takes the FULL inputs of reference.setup_inputs()
and returns the FULL [16, 1024, 768] float32 output.

Strategy: pure data parallelism — 16 batches / 8 cores = 2 batches per core,
weights replicated, zero collectives.  Per core a fused Bass/Tile program runs
LN1 -> QKV -> attention -> proj -> residual -> LN2 -> fc1/gelu -> fc2 ->
(normalized residual) -> LN3.

Precision plan: all large GEMMs use fp8e4 (e4m3) operands with
perf_mode=DoubleRow (two 128-deep k-tiles contracted per instruction), with
weights pre-scaled by 512 on the host so their magnitudes sit in fp8's
normal range; the 1/512 unscale is folded into the downstream bias/residual
ops (or the exp/gelu activation scale).  The attention scores q@k run in
bf16 (qkT tile is bf16).  LayerNorms, softmax denominators, residual adds
run in fp32/bf16 on DVE/ACT/Pool.  Host-emulated end-to-end error of this
scheme is ~1.1e-2 max-rel vs the fp32 reference (gate: 2e-2).

Engine budget per core (cost model): PE ~215us (matmuls), ACT ~260us
(softmax exp + gelu + LN sqrt; exp is ACT-only on TRN2), DVE ~200us (all
PSUM evacuation - GpSimd has no PSUM port), Pool ~120us (SBUF-side scales,
pre-adds, final writes).
"""

import os
import sys

sys.path.insert(0, "/opt/trn_rl_repo")
# The axon NTFF profiling hook is unavailable in this environment; force
# tracing off so an externally-set BASS_TRACE cannot break execution.
os.environ["BASS_NEVER_TRACE"] = "1"

import numpy as np
import ml_dtypes

import concourse.bass as bass
import concourse.tile as tile
from concourse import mybir
from concourse.masks import make_identity
from concourse.vector_clock import ScopedClock, VectorClock
from concourse.bass_utils import run_bass_kernel_spmd

F32 = mybir.dt.float32
BF = mybir.dt.bfloat16
F8 = mybir.dt.float8e4
AF = mybir.ActivationFunctionType
ALU = mybir.AluOpType
DR = mybir.MatmulPerfMode.DoubleRow

B, N, D = 16, 1024, 768
H, DH, HID = 12, 64, 3072
NCORES = 8
BLOC = B // NCORES
EPS = 1e-5
TC_N = N // 128   # 8 token tiles / batch
KC_D = D // 128   # 6 feature chunks
MC_H = HID // 128  # 24 hidden chunks
SW = 512.0        # host weight prescale (folded out after each GEMM)
SC = 32.0         # ctx scale, carried by the bb broadcast row value

E4NP = ml_dtypes.float8_e4m3


# ---------------------------------------------------------------------------
# Workarounds: this walrus build rejects >1 sync-wait command per instruction.
# ---------------------------------------------------------------------------
def _patched_drain_and_barrier(self, tick_clock, wait_clock):
    gc = tick_clock.global_clock
    n = len(gc)
    for i in range(n):
        t = gc[i]
        if t <= 0:
            continue
        vec = [0] * n
        vec[i] = t
        nop = self.nc.sync.nop(nofuse=True)
        wait_clock.add_sem_waits(nop.ins, ScopedClock({None: VectorClock(vec)}))
    self.nc.sync.drain()
    self.nc.all_engine_barrier()
    assert self.sems is not None
    popped = self.nc._tile_sem_poison_stack.pop()
    assert popped is self._sem_poison
    self.nc.clear_and_free_semaphores(list(self.sems.allocated().values()))
    self.nc.all_engine_barrier()


tile.TileContext._drain_and_barrier = _patched_drain_and_barrier


def _split_sync_waits(nc, limit=1):
    """Move excess per-instruction sync waits onto same-engine NoOps."""
    n_split = 0
    for fn in nc.m.functions:
        for bb in fn.blocks:
            out = []
            for ins in bb.instructions:
                si = ins.sync_info
                waits = list(si.on_wait) if (si and si.on_wait) else []
                if len(waits) > limit:
                    excess, keep = waits[:-limit], waits[-limit:]
                    for w in excess:
                        nop = mybir.InstNoOp(
                            name=f"{ins.name}-ws{n_split}",
                            engine=ins.engine,
                            ins=[],
                            outs=[],
                            sync_info=mybir.SyncInfo(on_wait=[w], on_update=[]),
                        )
                        n_split += 1
                        out.append(nop)
                    si.on_wait = keep
                out.append(ins)
            bb.instructions = out
    return n_split


# ---------------------------------------------------------------------------
# Device program (one core's share: BLOC batches)
# ---------------------------------------------------------------------------
def _layer_norm(nc, misc, x_t, eps_t):
    """(mean, rstd) [128,1] via DVE bn_stats for x_t [128, 768]."""
    xr = x_t.rearrange("p (s d) -> p s d", d=256)
    lt = misc.tile([128, 24], F32, tag="lnb")
    stats = lt[:, 0:18].rearrange("p (s d) -> p s d", d=6)
    for s in range(3):
        nc.vector.bn_stats(out=stats[:, s, :], in_=xr[:, s, :])
    mv = lt[:, 18:20]
    nc.vector.bn_aggr(out=mv, in_=lt[:, 0:18])
    std = lt[:, 20:21]
    nc.scalar.activation(out=std, in_=mv[:, 1:2], func=AF.Sqrt, bias=eps_t)
    rstd = misc.tile([128, 1], F32, tag="rstd")
    nc.vector.reciprocal(out=rstd, in_=std)
    return mv[:, 0:1], rstd


def _build_nc(reps=1, skip_gb2=False, skip_gb3=False):
    nc = bass.Bass()

    x_d = nc.dram_tensor("x", [BLOC, N, D], F32, kind="ExternalInput")
    wqk_d = nc.dram_tensor("w_qk", [D, 2 * D], F8, kind="ExternalInput")
    wv_d = nc.dram_tensor("w_v", [D, D], F8, kind="ExternalInput")
    bqk_d = nc.dram_tensor("b_qk", [2 * D], F32, kind="ExternalInput")
    bv_d = nc.dram_tensor("b_v", [D], F32, kind="ExternalInput")
    pw_d = nc.dram_tensor("p_w", [D, D], F8, kind="ExternalInput")
    pbr_d = nc.dram_tensor("pbr", [D], F32, kind="ExternalInput")
    w1_d = nc.dram_tensor("w1", [D, HID], F8, kind="ExternalInput")
    b1_d = nc.dram_tensor("b1", [HID], F32, kind="ExternalInput")
    w2_d = nc.dram_tensor("w2", [HID, D], F8, kind="ExternalInput")
    b2_d = nc.dram_tensor("b2", [D], F32, kind="ExternalInput")
    g2_d = nc.dram_tensor("g2", [D], F32, kind="ExternalInput")
    bt2_d = nc.dram_tensor("bt2", [D], F32, kind="ExternalInput")
    g3_d = nc.dram_tensor("g3", [D], F32, kind="ExternalInput")
    bt3_d = nc.dram_tensor("bt3", [D], F32, kind="ExternalInput")
    y_d = nc.dram_tensor("y", [BLOC, N, D], F32, kind="ExternalOutput")

    with tile.TileContext(nc, pool_alloc_mode="queue") as tc:
        misc = tc.alloc_tile_pool(name="misc", bufs=2)
        const = tc.alloc_tile_pool(name="const", bufs=1)

        identb = const.tile([128, 128], BF)
        make_identity(nc, identb)
        eps_t = const.tile([128, 1], F32)
        nc.vector.memset(eps_t, EPS)
        row32 = const.tile([1, 128], BF)
        nc.vector.memset(row32, SC)
        bqk_t = const.tile([128, 12], F32)
        nc.sync.dma_start(out=bqk_t, in_=bqk_d.rearrange("(c p) -> p c", p=128))
        b1_t = const.tile([128, MC_H], F32)
        nc.sync.dma_start(out=b1_t, in_=b1_d.rearrange("(c p) -> p c", p=128))

        def load_bc(dd, nm):
            t = const.tile([128, D], F32, name=nm)
            nc.sync.dma_start(out=t, in_=dd[None, :].partition_broadcast(128))
            return t

        bv_bc = load_bc(bv_d, "bv_bc")
        pbr_bc = load_bc(pbr_d, "pbr_bc")
        b2_bc = load_bc(b2_d, "b2_bc")
        if not skip_gb2:
            g2_bc = load_bc(g2_d, "g2_bc")
            bt2_bc = load_bc(bt2_d, "bt2_bc")
        if not skip_gb3:
            g3_bc = load_bc(g3_d, "g3_bc")
            bt3_bc = load_bc(bt3_d, "bt3_bc")

        # --- weights, loaded once, fp8, pre-scaled by SW on the host ---
        wqk = const.tile([128, KC_D, 2 * D], F8)
        nc.gpsimd.dma_start(out=wqk, in_=wqk_d.rearrange("(c p) n -> p c n", p=128))
        wv = const.tile([128, KC_D, D], F8)
        nc.gpsimd.dma_start(out=wv, in_=wv_d.rearrange("(c p) n -> p c n", p=128))
        pw = const.tile([128, KC_D, D], F8)
        nc.gpsimd.dma_start(out=pw, in_=pw_d.rearrange("(c p) n -> p c n", p=128))
        w1t = const.tile([128, KC_D, HID], F8)
        nc.gpsimd.dma_start(out=w1t, in_=w1_d.rearrange("(c p) n -> p c n", p=128))
        w2t = const.tile([128, MC_H, D], F8)
        nc.gpsimd.dma_start(out=w2t, in_=w2_d.rearrange("(c p) n -> p c n", p=128))

        # --- persistent activations tiles (reused across the 2 batches) ---
        xnT = const.tile([128, KC_D, N], F8)       # ln1(x)^T
        qkT = const.tile([128, 12, N], F8)         # q,k feature-major
        ctxT = const.tile([128, KC_D, N], F8)      # attention ctx^T (x SC)
        ffinT = const.tile([128, KC_D, N], F8)     # ln2 out^T
        ffhT = const.tile([128, MC_H, N], F8)      # gelu(fc1)^T
        ffin_r = const.tile([128, TC_N, D], BF)    # ln2 out (residual)
        # V1: per (token-chunk, head): even head -> V cols 0:64, ones col 64;
        # odd head -> ones col 0, V cols 64:128.  The softmax denominator
        # rides along the PV matmul on the aligned spare partition.
        V1 = const.tile([128, TC_N, H, 128], F8)
        nc.gpsimd.memset(V1[:, :, 0::2, 65:128], 0.0)
        nc.gpsimd.memset(V1[:, :, 1::2, 1:64], 0.0)
        nc.gpsimd.memset(V1[:, :, 0::2, 64:65], 1.0)
        nc.gpsimd.memset(V1[:, :, 1::2, 0:1], 1.0)

        bv3 = bv_bc.rearrange("p (h d) -> p h d", d=DH)

        for b in range(BLOC):
            # ---- Phase A: LN1 -> xn (bf16) -> PE transpose -> xnT (fp8) ----
            psA = tc.alloc_tile_pool(name=f"psA{b}", bufs=6, space="PSUM")
            for tcn in range(TC_N):
                x_t = misc.tile([128, D], F32, tag="x_in", bufs=3)
                nc.sync.dma_start(out=x_t, in_=x_d[b, tcn * 128:(tcn + 1) * 128, :])
                mean, rstd = _layer_norm(nc, misc, x_t, eps_t)
                xn_bf = misc.tile([128, D], BF, tag="xn_bf", bufs=3)
                nc.vector.tensor_scalar(out=xn_bf, in0=x_t, scalar1=mean,
                                        scalar2=rstd, op0=ALU.subtract,
                                        op1=ALU.mult)
                for kc in range(KC_D):
                    pt = psA.tile([128, 128], BF, tag="tp")
                    nc.tensor.transpose(pt, xn_bf[:, kc * 128:(kc + 1) * 128],
                                        identb)
                    nc.vector.tensor_copy(
                        out=xnT[:, kc, tcn * 128:(tcn + 1) * 128], in_=pt)
            psA.release()

            # ---- Phase B: Q,K projection (feature-major, bf16 out) ----
            psB = tc.alloc_tile_pool(name=f"psB{b}", bufs=3, space="PSUM")
            for fc in range(12):
                for nh in range(2):
                    ps = psB.tile([128, 512], F32, tag="qk")
                    for k2 in range(KC_D // 2):
                        nc.tensor.matmul(
                            ps,
                            wqk[:, 2 * k2:2 * k2 + 2, fc * 128:(fc + 1) * 128],
                            xnT[:, 2 * k2:2 * k2 + 2, nh * 512:(nh + 1) * 512],
                            start=(k2 == 0), stop=(k2 == KC_D // 2 - 1),
                            perf_mode=DR)
                    nc.vector.tensor_scalar(
                        out=qkT[:, fc, nh * 512:(nh + 1) * 512], in0=ps,
                        scalar1=1.0 / SW, scalar2=bqk_t[:, fc:fc + 1],
                        op0=ALU.mult, op1=ALU.add)
            psB.release()

            # ---- Phase C: V projection (token-major, packed [V_h | ones]) ----
            psC = tc.alloc_tile_pool(name=f"psC{b}", bufs=2, space="PSUM")
            for tcn in range(TC_N):
                ps = psC.tile([128, D], F32, tag="v")
                lhs = xnT[:, :, tcn * 128:(tcn + 1) * 128]
                for k2 in range(KC_D // 2):
                    nc.tensor.matmul(ps[:, 0:512],
                                     lhs[:, 2 * k2:2 * k2 + 2, :],
                                     wv[:, 2 * k2:2 * k2 + 2, 0:512],
                                     start=(k2 == 0), stop=(k2 == 2),
                                     perf_mode=DR)
                    nc.tensor.matmul(ps[:, 512:768],
                                     lhs[:, 2 * k2:2 * k2 + 2, :],
                                     wv[:, 2 * k2:2 * k2 + 2, 512:768],
                                     start=(k2 == 0), stop=(k2 == 2),
                                     perf_mode=DR)
                pv3 = ps.rearrange("p (h d) -> p h d", d=DH)
                nc.vector.scalar_tensor_tensor(
                    out=V1[:, tcn, 0::2, 0:64], in0=pv3[:, 0::2, :],
                    scalar=1.0 / SW, in1=bv3[:, 0::2, :],
                    op0=ALU.mult, op1=ALU.add)
                nc.vector.scalar_tensor_tensor(
                    out=V1[:, tcn, 1::2, 64:128], in0=pv3[:, 1::2, :],
                    scalar=1.0 / SW, in1=bv3[:, 1::2, :],
                    op0=ALU.mult, op1=ALU.add)
            psC.release()

            # ---- Phase D: attention ----
            pPT = tc.alloc_tile_pool(name=f"pPT{b}", bufs=2)
            psS = tc.alloc_tile_pool(name=f"psS{b}", bufs=2, space="PSUM")
            psX = tc.alloc_tile_pool(name=f"psX{b}", bufs=2, space="PSUM")
            for nh in range(2):
                for h in range(H):
                    qh = (h % 2) * 64
                    dh = 64 if h % 2 == 0 else 0  # denominator partition
                    fq, fk = h // 2, 6 + h // 2
                    PT = pPT.tile([128, TC_N, 512], F8, tag="PT", bufs=2)
                    for mc2 in range(TC_N // 2):
                        sp = psS.tile([128, 1024], F32, tag="sc")
                        for mi in range(2):
                            mc = mc2 * 2 + mi
                            nc.tensor.matmul(
                                sp[:, mi * 512:(mi + 1) * 512],
                                qkT[qh:qh + 64, fk, mc * 128:(mc + 1) * 128],
                                qkT[qh:qh + 64, fq, nh * 512:(nh + 1) * 512],
                                start=True, stop=True)
                        nc.scalar.activation(
                            out=PT[:, mc2 * 2:mc2 * 2 + 2, :], in_=sp,
                            func=AF.Exp, scale=0.125)
                    cp = psX.tile([128, 512], F32, tag="cp")
                    for m2 in range(TC_N // 2):
                        nc.tensor.matmul(cp, V1[:, 2 * m2:2 * m2 + 2, h, :],
                                         PT[:, 2 * m2:2 * m2 + 2, :],
                                         start=(m2 == 0), stop=(m2 == 3),
                                         perf_mode=DR)
                    rd = misc.tile([1, 512], BF, tag="rd", name="rd")
                    with nc.allow_low_precision(reason="bf16 recip"):
                        nc.vector.reciprocal(out=rd, in_=cp[dh:dh + 1, :])
                    bb = psX.tile([128, 512], F32, tag="bb")
                    nc.tensor.matmul(bb, row32, rd, start=True, stop=True)
                    nc.vector.tensor_tensor(
                        out=ctxT[qh:qh + 64, fq, nh * 512:(nh + 1) * 512],
                        in0=cp[qh:qh + 64, :], in1=bb[qh:qh + 64, :],
                        op=ALU.mult)
            psX.release()
            psS.release()
            pPT.release()

            # ---- Phase E: proj + residual -> x2 -> LN2 -> ffin + ffinT ----
            psE = tc.alloc_tile_pool(name=f"psE{b}", bufs=2, space="PSUM")
            psT = tc.alloc_tile_pool(name=f"psT{b}", bufs=4, space="PSUM")
            for tcn in range(TC_N):
                ps = psE.tile([128, D], F32, tag="pj")
                x_t = misc.tile([128, D], F32, tag="x_in", bufs=3)
                nc.sync.dma_start(out=x_t, in_=x_d[b, tcn * 128:(tcn + 1) * 128, :])
                xpb = misc.tile([128, D], F32, tag="xpb", bufs=3)
                nc.gpsimd.tensor_tensor(out=xpb, in0=x_t, in1=pbr_bc, op=ALU.add)
                lhs = ctxT[:, :, tcn * 128:(tcn + 1) * 128]
                for k2 in range(KC_D // 2):
                    nc.tensor.matmul(ps[:, 0:512],
                                     lhs[:, 2 * k2:2 * k2 + 2, :],
                                     pw[:, 2 * k2:2 * k2 + 2, 0:512],
                                     start=(k2 == 0), stop=(k2 == 2),
                                     perf_mode=DR)
                    nc.tensor.matmul(ps[:, 512:768],
                                     lhs[:, 2 * k2:2 * k2 + 2, :],
                                     pw[:, 2 * k2:2 * k2 + 2, 512:768],
                                     start=(k2 == 0), stop=(k2 == 2),
                                     perf_mode=DR)
                x2 = misc.tile([128, D], F32, tag="xwork", bufs=3)
                nc.vector.scalar_tensor_tensor(
                    out=x2, in0=ps, scalar=1.0 / (SW * SC), in1=xpb,
                    op0=ALU.mult, op1=ALU.add)
                mean, rstd = _layer_norm(nc, misc, x2, eps_t)
                fi = ffin_r[:, tcn, :]
                nc.vector.tensor_scalar(out=fi, in0=x2, scalar1=mean,
                                        scalar2=rstd, op0=ALU.subtract,
                                        op1=ALU.mult)
                if not skip_gb2:
                    nc.gpsimd.tensor_tensor(out=fi, in0=fi, in1=g2_bc, op=ALU.mult)
                    nc.gpsimd.tensor_tensor(out=fi, in0=fi, in1=bt2_bc, op=ALU.add)
                for kc in range(KC_D):
                    pt = psT.tile([128, 128], BF, tag="tp2")
                    nc.tensor.transpose(pt, fi[:, kc * 128:(kc + 1) * 128],
                                        identb)
                    nc.vector.tensor_copy(
                        out=ffinT[:, kc, tcn * 128:(tcn + 1) * 128], in_=pt)
            psT.release()
            psE.release()

            # ---- Phase F: fc1 + gelu -> ffhT (fp8) ----
            psF = tc.alloc_tile_pool(name=f"psF{b}", bufs=2, space="PSUM")
            for mc in range(MC_H):
                ps = psF.tile([128, N], F32, tag="f1")
                for k2 in range(KC_D // 2):
                    w1s = w1t[:, 2 * k2:2 * k2 + 2, mc * 128:(mc + 1) * 128]
                    for nh in range(2):
                        nc.tensor.matmul(
                            ps[:, nh * 512:(nh + 1) * 512],
                            w1s,
                            ffinT[:, 2 * k2:2 * k2 + 2, nh * 512:(nh + 1) * 512],
                            start=(k2 == 0), stop=(k2 == 2),
                            perf_mode=DR)
                nc.scalar.activation(out=ffhT[:, mc, :], in_=ps, func=AF.Gelu,
                                     bias=b1_t[:, mc:mc + 1], scale=1.0 / SW)
            psF.release()

            # ---- Phase G: fc2 + residual + LN3 -> y ----
            psG = tc.alloc_tile_pool(name=f"psG{b}", bufs=2, space="PSUM")
            for ti in range(TC_N):
                ps = psG.tile([128, D], F32, tag="f2")
                lhs = ffhT[:, :, ti * 128:(ti + 1) * 128]
                for k2 in range(MC_H // 2):
                    nc.tensor.matmul(ps[:, 0:512],
                                     lhs[:, 2 * k2:2 * k2 + 2, :],
                                     w2t[:, 2 * k2:2 * k2 + 2, 0:512],
                                     start=(k2 == 0), stop=(k2 == 11),
                                     perf_mode=DR)
                    nc.tensor.matmul(ps[:, 512:768],
                                     lhs[:, 2 * k2:2 * k2 + 2, :],
                                     w2t[:, 2 * k2:2 * k2 + 2, 512:768],
                                     start=(k2 == 0), stop=(k2 == 11),
                                     perf_mode=DR)
                x3 = misc.tile([128, D], F32, tag="xwork", bufs=3)
                nc.vector.scalar_tensor_tensor(
                    out=x3, in0=ps, scalar=1.0 / SW, in1=ffin_r[:, ti, :],
                    op0=ALU.mult, op1=ALU.add)
                nc.gpsimd.tensor_tensor(out=x3, in0=x3, in1=b2_bc, op=ALU.add)
                mean, rstd = _layer_norm(nc, misc, x3, eps_t)
                yt = misc.tile([128, D], F32, tag="xout", bufs=3)
                neng = nc.vector if ti % 2 == 0 else nc.gpsimd
                neng.tensor_scalar(out=yt, in0=x3, scalar1=mean,
                                   scalar2=rstd,
                                   op0=ALU.subtract, op1=ALU.mult)
                if not skip_gb3:
                    nc.gpsimd.tensor_tensor(out=yt, in0=yt, in1=g3_bc, op=ALU.mult)
                    nc.gpsimd.tensor_tensor(out=yt, in0=yt, in1=bt3_bc, op=ALU.add)
                nc.sync.dma_start(out=y_d[b, ti * 128:(ti + 1) * 128, :], in_=yt)
            psG.release()

        const.release()
        misc.release()

    _split_sync_waits(nc)
    return nc


_NC_CACHE = {}


def _get_nc(reps=1, skip_gb2=False, skip_gb3=False):
    key = f"nc{reps}_{skip_gb2}_{skip_gb3}"
    if key not in _NC_CACHE:
        _NC_CACHE[key] = _build_nc(reps, skip_gb2, skip_gb3)
    return _NC_CACHE[key]


def kernel(x, ln1_g, ln1_b, qkv_w, qkv_b, proj_w, proj_b,
           ln2_g, ln2_b, fc1_w, fc1_b, fc2_w, fc2_b, ln3_g, ln3_b,
           **extra):
    x = np.ascontiguousarray(np.asarray(x, np.float32))
    f = lambda a: np.ascontiguousarray(np.asarray(a, np.float32))
    f8 = lambda a: np.ascontiguousarray(np.asarray(SW * a, E4NP))
    qkv_w, qkv_b = f(qkv_w), f(qkv_b)

    # Fold LN1 gamma/beta into QKV weights/bias (host, fp32).
    w_eff = np.asarray(ln1_g, np.float32)[:, None] * qkv_w
    b_eff = np.asarray(ln1_b, np.float32) @ qkv_w + qkv_b

    common = {
        "w_qk": f8(w_eff[:, :2 * D]),
        "w_v": f8(w_eff[:, 2 * D:]),
        "b_qk": f(b_eff[:2 * D]),
        "b_v": f(b_eff[2 * D:]),
        "p_w": f8(proj_w), "pbr": f(proj_b),
        "w1": f8(fc1_w), "b1": f(fc1_b),
        "w2": f8(fc2_w), "b2": f(fc2_b),
        "g2": f(ln2_g), "bt2": f(ln2_b),
        "g3": f(ln3_g), "bt3": f(ln3_b),
    }
    in_maps = [dict(common, x=x[i * BLOC:(i + 1) * BLOC]) for i in range(NCORES)]

    skip_gb2 = bool(np.all(common["g2"] == 1.0) and np.all(common["bt2"] == 0.0))
    skip_gb3 = bool(np.all(common["g3"] == 1.0) and np.all(common["bt3"] == 0.0))
    nc = _get_nc(1, skip_gb2, skip_gb3)
    res = run_bass_kernel_spmd(nc, in_maps, core_ids=list(range(NCORES)))
    _NC_CACHE["last_result"] = res
    return np.concatenate([r["y"] for r in res.results], axis=0)
